# revision 28
# baseline (speedup 1.0000x reference)
"""Trainium2 Bass kernel for nn_BestHits: out = bh * bh.T where
bh = blockwise-softmax(mask_diag(similarities) / TAU) over 256-wide column groups.

Strategy (v5, "host-transposed-B"): out is symmetric, so only the 136
upper-incl-diagonal 512x512 block-pairs are computed (17 per core on 8 cores);
out[J,I] = out[I,J].T is mirrored on the host.

The host stages, per pair (I, J): A = sims[I,J] (fp16) and BT = sims[J,I].T
(fp16, pre-transposed for free on the CPU).  With B already transposed on
device there are NO transpose matmuls and no PSUM-resident product:

  zab   = exp([A || BT] / TAU)            one fused ACT instr, bf16 out
  sb    = column-group sums of B          = partition-group sums of zbt -> PE
          (ones[128,1].T @ zbt subtiles accumulated in a [2,512] PSUM row-pair)
  rbT   = 1/sb                            DVE reciprocal_approx_fast (51 ULP)
  rb16  = bf16(rbT)                       ACT copy (ScE is near PSUM/idle-ish)
  RBb   = broadcast rbT to 128 partitions GPSIMD partition_broadcast
  sa    = row-group sums of A             GPSIMD tensor_reduce (free-dim, 1x)
  ra    = 1/sa                            DVE reciprocal (tiny [P,8])
  w1    = zbt * RBb   (= bhB.T, <=1)      DVE TT bf16 2x-mode
  w2    = w1 * za                         DVE TT bf16 2x-mode
  out   = w2 * ra  per (t, colgroup)      8x DVE tensor_scalar 4x-mode
(w1 is normalized first so no bf16 overflow: za*zbt alone can reach e^110.)

Engine budget per slot (ns, modeled): ACT 4330, DVE 4910, Pool 4660, PE 850,
Sync ~2500 (2 loads + 1 store), DMA 1.5 MiB HBM.  fp16 in / bf16 out halves
HBM traffic vs fp32: 25.5 MiB/core -> ~68 us DMA floor.
"""
import sys

import numpy as np

sys.path.insert(0, "/opt/trn_rl_repo")

from contextlib import ExitStack

import concourse.bass as bass  # noqa: F401  (registers AP machinery)
import concourse.tile as tile
from concourse import bacc, mybir
from concourse.bass_utils import run_bass_kernel_spmd

N = 8192          # full matrix side
B = 512           # block side
NB = N // B       # 16 blocks per side
P = 128           # SBUF partitions
T = B // P        # 4 row-subtiles per block
GRP = 256         # softmax group width
NG = B // GRP     # 2 groups per block side
TAU = 0.1
NDIAG = 2         # diagonal pairs per core (the last NDIAG slots)
NSLOTS = 17       # block-pairs per core
NCORES = 8
MASK = -60000.0   # diagonal mask; representable in fp16, exp(MASK/TAU) == 0

F32 = mybir.dt.float32
F16 = mybir.dt.float16
BF16 = mybir.dt.bfloat16
AF = mybir.ActivationFunctionType
OP = mybir.AluOpType


def core_pairs() -> list[list[tuple[int, int]]]:
    """136 upper-triangle block pairs distributed 17-per-core; the 2 diagonal
    pairs of each core come last (kept for layout compat; slots are uniform)."""
    diag = [(i, i) for i in range(NB)]
    off = [(i, j) for i in range(NB) for j in range(i + 1, NB)]
    cps: list[list[tuple[int, int]]] = [[] for _ in range(NCORES)]
    for idx, p in enumerate(off):
        cps[idx % NCORES].append(p)
    for idx, p in enumerate(diag):
        cps[idx % NCORES].append(p)
    return cps


CORE_PAIRS = core_pairs()


def build():
    """Build + compile the (single-program, 8-core SPMD) Bass kernel."""
    nc = bacc.Bacc(
        "TRN2",
        target_bir_lowering=False,
        debug=False,
        enable_asserts=True,
        num_devices=NCORES,
    )
    ab = nc.dram_tensor("ab", [NSLOTS, P, 2, T, B], F16, kind="ExternalInput").ap()
    selc = nc.dram_tensor("selc", [NG, NG, P], BF16, kind="ExternalInput").ap()
    o = nc.dram_tensor("o", [NSLOTS, P, T, B], BF16, kind="ExternalOutput").ap()

    with tile.TileContext(nc) as tc, ExitStack() as ctx:
        const_pool = ctx.enter_context(tc.tile_pool(name="const", bufs=1))
        # e[g]: [P, 2] indicator column g — the ones.T @ zbt group-sum matmul
        # lands group g's column sums on PSUM row g.
        es = []
        for g in range(NG):
            e = const_pool.tile([P, NG], BF16, name=f"e{g}")
            nc.vector.memset(e[:], 0.0)
            nc.vector.memset(e[:, g:g + 1], 1.0)
            es.append(e)
        # sel[g]: [2, 128] row-selector — rank-1 matmul sel[g].T @ rb16
        # replicates rb16 row g onto all 128 output partitions (a broadcast
        # done entirely on the PE; contraction dim 2, base partition 0).
        sels = []
        for g in range(NG):
            s = const_pool.tile([NG, P], BF16, name=f"sel{g}")
            nc.sync.dma_start(s[:], selc[g])
            sels.append(s)

        src_pool = ctx.enter_context(tc.tile_pool(name="src", bufs=4))
        zab_pool = ctx.enter_context(tc.tile_pool(name="zab", bufs=4))
        w1_pool = ctx.enter_context(tc.tile_pool(name="w1", bufs=3))
        w2_pool = ctx.enter_context(tc.tile_pool(name="w2", bufs=3))
        o_pool = ctx.enter_context(tc.tile_pool(name="o_sb", bufs=3))
        st_pool = ctx.enter_context(tc.tile_pool(name="st", bufs=5))
        ps_pool = ctx.enter_context(tc.tile_pool(name="ps", bufs=3, space="PSUM"))
        pb_pool = ctx.enter_context(tc.tile_pool(name="pb", bufs=2, space="PSUM"))

        for k in range(NSLOTS):
            # --- one load: A (half 0) and BT (half 1) in one fp16 tile ------
            src = src_pool.tile([P, 2, T, B], F16)
            nc.sync.dma_start(src[:], ab[k])

            # --- one fused exp over both blocks, bf16 out -------------------
            zab = zab_pool.tile([P, 2, T, B], BF16)
            nc.scalar.activation(zab[:], src[:], AF.Exp, scale=1.0 / TAU)
            za = zab[:, 0]    # [P, T, B] = exp(A)
            zbt = zab[:, 1]   # [P, T, B] = exp(B.T) = exp(B).T

            # --- B-side group sums on PE: sb[g, c] = sum_{r in g} zbt[r, c] -
            # (zbt rows are B's columns, so partition-group sums over two
            # 128-row subtiles give the 256-wide column-group sums of B.)
            ps = ps_pool.tile([NG, B], F32)
            for t in range(T):
                nc.tensor.matmul(
                    ps[:, :], es[t // 2][:], zbt[:, t, :],
                    start=(t == 0), stop=(t == T - 1),
                )

            # --- rbT = 1/sb: DVE recip, ACT bf16, PE-broadcast, ACT drain ---
            rf32 = st_pool.tile([NG, B], F32, name="rf32")
            nc.vector.reciprocal_approx_fast(out=rf32[:], in_=ps[:])
            rb16 = st_pool.tile([NG, B], BF16, name="rb16")
            nc.scalar.copy(rb16[:], rf32[:])
            rbps = pb_pool.tile([P, NG, B], F32)
            for g in range(NG):
                nc.tensor.matmul(
                    rbps[:, g, :], sels[g][:], rb16[:],
                    start=True, stop=True,
                )
            rbb = st_pool.tile([P, NG, B], BF16, name="rbb")
            nc.scalar.copy(rbb[:], rbps[:])

            # --- A-side group sums (two-level bf16 tree + reduce), recip ----
            zs = st_pool.tile([P, T * NG, GRP // 2], BF16, name="zs")
            zav = za.rearrange("p t b -> p (t b)").rearrange(
                "p (G two s) -> p G two s", two=2, s=GRP // 2
            )
            nc.vector.tensor_tensor(zs[:], zav[:, :, 0], zav[:, :, 1], op=OP.add)
            zs2 = st_pool.tile([P, T * NG, GRP // 4], BF16, name="zs2")
            zsv = zs[:].rearrange("p G (two s) -> p G two s", two=2)
            nc.vector.tensor_tensor(zs2[:], zsv[:, :, 0], zsv[:, :, 1], op=OP.add)
            sa = st_pool.tile([P, T * NG], F32, name="sa")
            nc.vector.tensor_reduce(
                sa[:], zs2[:], axis=mybir.AxisListType.X, op=OP.add
            )
            ra = st_pool.tile([P, T * NG], F32, name="ra")
            nc.vector.reciprocal(ra[:], sa[:])
            # expand ra -> [P, 8, GRP] bf16 via a stride-0 copy fan-out chain
            # (steps 2 and 3 have packed inner runs, so they hit fast modes)
            r2 = st_pool.tile([P, T * NG, 2], BF16, name="r2")
            nc.vector.tensor_copy(
                r2[:],
                ra[:].rearrange("p (G one) -> p G one", one=1)
                .broadcast_to([P, T * NG, 2]),
            )
            raw = st_pool.tile([P, T * NG, GRP], BF16, name="raw")
            nc.vector.tensor_copy(
                raw[:].rearrange("p G (f r) -> p G f r", r=2),
                r2[:].rearrange("p G (one r) -> p G one r", one=1)
                .broadcast_to([P, T * NG, GRP // 2, 2]),
            )

            # --- product: w1 = bhB.T (<=1), w2 = w1*za, out = w2*raw --------
            w1 = w1_pool.tile([P, T, B], BF16)
            nc.vector.tensor_tensor(
                w1[:].rearrange("p (g u) b -> p g u b", g=NG),
                zbt.rearrange("p (g u) b -> p g u b", g=NG),
                rbb[:].rearrange("p g (one b) -> p g one b", one=1)
                .broadcast_to([P, NG, T // NG, B]),
                op=OP.mult,
            )
            w2 = w2_pool.tile([P, T, B], BF16)
            nc.vector.tensor_tensor(w2[:], w1[:], za, op=OP.mult)
            o_sb = o_pool.tile([P, T, B], BF16)
            nc.vector.tensor_tensor(
                o_sb[:].rearrange("p t b -> p (t b)"),
                w2[:].rearrange("p t b -> p (t b)"),
                raw[:].rearrange("p G s -> p (G s)"),
                op=OP.mult,
            )
            nc.sync.dma_start(o[k], o_sb[:])

    nc.compile()
    return nc


_NC = None


def _get_nc():
    global _NC
    if _NC is None:
        _NC = build()
    return _NC


def _to_pmajor(blocks: np.ndarray) -> np.ndarray:
    # (n, 512, 512) row-major -> (n, 128, 4, 512): row r = t*P + p lands at
    # [p, t, :], so every SBUF partition's bytes are contiguous in DRAM.
    n = blocks.shape[0]
    return np.ascontiguousarray(
        blocks.reshape(n, T, P, B).transpose(0, 2, 1, 3)
    )


def make_in_maps(sims: np.ndarray) -> list[dict[str, np.ndarray]]:
    in_maps = []
    for c in range(NCORES):
        a_stack = np.empty((NSLOTS, B, B), np.float16)
        bt_stack = np.empty((NSLOTS, B, B), np.float16)
        for k, (i, j) in enumerate(CORE_PAIRS[c]):
            ablk = sims[i * B:(i + 1) * B, j * B:(j + 1) * B].astype(np.float16)
            if i == j:
                np.fill_diagonal(ablk, MASK)
            a_stack[k] = ablk
            if i == j:
                bt_stack[k] = ablk.T
            else:
                bt_stack[k] = (
                    sims[j * B:(j + 1) * B, i * B:(i + 1) * B]
                    .astype(np.float16).T
                )
        from ml_dtypes import bfloat16

        sel = np.zeros((NG, NG, P), bfloat16)
        for g in range(NG):
            sel[g, g, :] = 1.0
        # ab[k, p, 0] = A rows, ab[k, p, 1] = B.T rows (partition-major)
        ab = np.ascontiguousarray(
            np.stack([_to_pmajor(a_stack), _to_pmajor(bt_stack)], axis=2)
        )
        in_maps.append({"ab": ab, "selc": sel})
    return in_maps


def assemble(results: list[dict[str, np.ndarray]]) -> np.ndarray:
    out = np.empty((N, N), np.float32)
    for c in range(NCORES):
        o_pm = np.asarray(results[c]["o"], dtype=np.float32)
        o_stack = np.ascontiguousarray(
            o_pm.transpose(0, 2, 1, 3).reshape(NSLOTS, B, B)
        )
        for k, (i, j) in enumerate(CORE_PAIRS[c]):
            out[i * B:(i + 1) * B, j * B:(j + 1) * B] = o_stack[k]
            if i != j:
                out[j * B:(j + 1) * B, i * B:(i + 1) * B] = o_stack[k].T
    return out


def run_on_hw(sims: np.ndarray, **spmd_kwargs):
    """Run the kernel on the 8 NeuronCores. Returns (out, BassKernelResults).

    The device occasionally throws a transient NRT_EXEC_UNIT_UNRECOVERABLE
    and needs ~a minute to come back, so failed runs are retried."""
    import time

    nc = _get_nc()
    in_maps = make_in_maps(sims)
    last_exc = None
    for attempt in range(3):
        if attempt:
            time.sleep(75)
        try:
            res = run_bass_kernel_spmd(
                nc, in_maps, core_ids=list(range(NCORES)), **spmd_kwargs
            )
            return assemble(res.results), res
        except Exception as exc:  # noqa: BLE001 - device flake, retry
            last_exc = exc
    raise last_exc


def kernel(similarities: np.ndarray) -> np.ndarray:
    sims = np.ascontiguousarray(similarities, dtype=np.float32)
    assert sims.shape == (N, N)
    out, _ = run_on_hw(sims)
    return out


if __name__ == "__main__":
    rng = np.random.default_rng(0)
    sims = rng.standard_normal((N, N), dtype=np.float32)
    out = kernel(similarities=sims)
    print("out", out.shape, out.dtype, float(out.max()))


# revision 31
# speedup vs baseline: 1.0313x; 1.0313x over previous
"""Trainium2 Bass kernel for nn_BestHits: out = bh * bh.T where
bh = blockwise-softmax(mask_diag(similarities) / TAU) over 256-wide column groups.

Strategy (v5, "host-transposed-B"): out is symmetric, so only the 136
upper-incl-diagonal 512x512 block-pairs are computed (17 per core on 8 cores);
out[J,I] = out[I,J].T is mirrored on the host.

The host stages, per pair (I, J), one fused fp16 tile holding A = sims[I,J]
and BT = sims[J,I].T (pre-transposed for free on the CPU).  With B already
transposed on device there are NO transpose matmuls and no PSUM-resident
product; the whole product pipeline runs on SBUF bf16 where the DVE gets its
2x/4x perf modes:

  zab  = exp([A || BT] / TAU)          one fused ACT instr, fp16 in, bf16 out
  sb   = column-group sums of B        = partition-group sums of zbt on the PE
         (indicator[P,2].T @ zbt subtiles -> [2,512] PSUM row-pair)
  rbT  = 1/sb                          DVE reciprocal_approx_fast (51 ULP)
  rb16 = bf16(rbT)                     ACT copy
  RBb  = rbT broadcast to 128 rows     PE rank-1: sel[2,128].T @ rb16 -> PSUM
         (gpsimd partition_broadcast corrupts tail bytes on HW - avoid)
  rbb  = bf16(RBb)                     ACT copy drains the PSUM broadcast
  sa   = row-group sums of A           DVE: 2-level bf16 pair-tree (TT 2x) +
                                       1x tensor_reduce of the [P,8,64] tail
  ra   = 1/sa                          DVE reciprocal (tiny [P,8])
  raw  = ra expanded to [P,8,256] bf16 DVE stride-0-source copy fan-out chain
  w1   = zbt * rbb  (= bhB.T, <=1)     DVE TT bf16 2x-mode
  w2   = w1 * za                       DVE TT bf16 2x-mode
  out  = w2 * raw                      DVE TT bf16 2x-mode
(w1 is normalized first so no bf16 overflow: za*zbt alone can reach e^110.)

Measured per-slot queue costs (ns): ACT exp 3693 + converts ~1800, DVE TTs
3x1200 + sums/recip/expand ~2900, PE 6 matmuls ~614 each, one 8 KiB/partition
load + one 4 KiB store on sync HWDGE.  fp16 in / bf16 out halves HBM traffic
vs fp32 (25.5 MiB/core).  HW: 138.2 us vs 156.6 us for the fp32 baseline.
"""
import sys

import numpy as np

sys.path.insert(0, "/opt/trn_rl_repo")

from contextlib import ExitStack

import concourse.bass as bass  # noqa: F401  (registers AP machinery)
import concourse.tile as tile
from concourse import bacc, mybir
from concourse.bass_utils import run_bass_kernel_spmd

N = 8192          # full matrix side
B = 512           # block side
NB = N // B       # 16 blocks per side
P = 128           # SBUF partitions
T = B // P        # 4 row-subtiles per block
GRP = 256         # softmax group width
NG = B // GRP     # 2 groups per block side
TAU = 0.1
NDIAG = 2         # diagonal pairs per core (the last NDIAG slots)
NSLOTS = 17       # block-pairs per core
NCORES = 8
MASK = -60000.0   # diagonal mask; representable in fp16, exp(MASK/TAU) == 0

F32 = mybir.dt.float32
F16 = mybir.dt.float16
BF16 = mybir.dt.bfloat16
AF = mybir.ActivationFunctionType
OP = mybir.AluOpType


def core_pairs() -> list[list[tuple[int, int]]]:
    """136 upper-triangle block pairs distributed 17-per-core; the 2 diagonal
    pairs of each core come last (kept for layout compat; slots are uniform)."""
    diag = [(i, i) for i in range(NB)]
    off = [(i, j) for i in range(NB) for j in range(i + 1, NB)]
    cps: list[list[tuple[int, int]]] = [[] for _ in range(NCORES)]
    for idx, p in enumerate(off):
        cps[idx % NCORES].append(p)
    for idx, p in enumerate(diag):
        cps[idx % NCORES].append(p)
    return cps


CORE_PAIRS = core_pairs()


def build():
    """Build + compile the (single-program, 8-core SPMD) Bass kernel."""
    nc = bacc.Bacc(
        "TRN2",
        target_bir_lowering=False,
        debug=False,
        enable_asserts=True,
        num_devices=NCORES,
    )
    ab = nc.dram_tensor("ab", [NSLOTS, P, 2, T, B], F16, kind="ExternalInput").ap()
    selc = nc.dram_tensor("selc", [NG, NG, P], BF16, kind="ExternalInput").ap()
    o = nc.dram_tensor("o", [NSLOTS, P, T, B], BF16, kind="ExternalOutput").ap()

    with tile.TileContext(nc) as tc, ExitStack() as ctx:
        const_pool = ctx.enter_context(tc.tile_pool(name="const", bufs=1))
        # e[g]: [P, 2] indicator column g — the ones.T @ zbt group-sum matmul
        # lands group g's column sums on PSUM row g.
        es = []
        for g in range(NG):
            e = const_pool.tile([P, NG], BF16, name=f"e{g}")
            nc.vector.memset(e[:], 0.0)
            nc.vector.memset(e[:, g:g + 1], 1.0)
            es.append(e)
        # sel[g]: [2, 128] row-selector — rank-1 matmul sel[g].T @ rb16
        # replicates rb16 row g onto all 128 output partitions (a broadcast
        # done entirely on the PE; contraction dim 2, base partition 0).
        sels = []
        for g in range(NG):
            s = const_pool.tile([NG, P], BF16, name=f"sel{g}")
            nc.sync.dma_start(s[:], selc[g])
            sels.append(s)

        src_pool = ctx.enter_context(tc.tile_pool(name="src", bufs=4))
        zab_pool = ctx.enter_context(tc.tile_pool(name="zab", bufs=5))
        w1_pool = ctx.enter_context(tc.tile_pool(name="w1", bufs=3))
        w2_pool = ctx.enter_context(tc.tile_pool(name="w2", bufs=3))
        o_pool = ctx.enter_context(tc.tile_pool(name="o_sb", bufs=3))
        st_pool = ctx.enter_context(tc.tile_pool(name="st", bufs=6))
        ps_pool = ctx.enter_context(tc.tile_pool(name="ps", bufs=3, space="PSUM"))
        pb_pool = ctx.enter_context(tc.tile_pool(name="pb", bufs=2, space="PSUM"))

        for k in range(NSLOTS):
            # --- one load: A (half 0) and BT (half 1) in one fp16 tile ------
            src = src_pool.tile([P, 2, T, B], F16)
            nc.sync.dma_start(src[:], ab[k])

            # --- one fused exp over both blocks, bf16 out -------------------
            zab = zab_pool.tile([P, 2, T, B], BF16)
            nc.scalar.activation(zab[:], src[:], AF.Exp, scale=1.0 / TAU)
            za = zab[:, 0]    # [P, T, B] = exp(A)
            zbt = zab[:, 1]   # [P, T, B] = exp(B.T) = exp(B).T

            # --- B-side group sums on PE: sb[g, c] = sum_{r in g} zbt[r, c] -
            # (zbt rows are B's columns, so partition-group sums over two
            # 128-row subtiles give the 256-wide column-group sums of B.)
            ps = ps_pool.tile([NG, B], F32)
            for t in range(T):
                nc.tensor.matmul(
                    ps[:, :], es[t // 2][:], zbt[:, t, :],
                    start=(t == 0), stop=(t == T - 1),
                )

            # --- rbT = 1/sb: one DVE approx-recip writing bf16 directly -----
            # (the bass wrapper insists on fp32 out, but only the INPUT needs
            # the fp32 bit layout for the BITWISE_NOT seed; the DVE write port
            # casts the final NR result to bf16 — saves the ACT convert.)
            from concourse.dve_ops import (
                RECIP_APPROX_FAST_CONSTS as RC,
                RECIPROCAL_APPROX_FAST,
            )

            rb16 = st_pool.tile([NG, B], BF16, name="rb16")
            nc.vector._custom_dve(
                RECIPROCAL_APPROX_FAST, out=rb16[:], in0=ps[:],
                s0=RC["s0"], s1=RC["s1"], imm2=RC["imm2"],
            )
            rbps = pb_pool.tile([P, NG, B], F32)
            for g in range(NG):
                nc.tensor.matmul(
                    rbps[:, g, :], sels[g][:], rb16[:],
                    start=True, stop=True,
                )
            rbb = st_pool.tile([P, NG, B], BF16, name="rbb")
            nc.scalar.copy(rbb[:], rbps[:])

            # --- A-side group sums (two-level bf16 tree + reduce), recip ----
            zs = st_pool.tile([P, T * NG, GRP // 2], BF16, name="zs")
            zav = za.rearrange("p t b -> p (t b)").rearrange(
                "p (G two s) -> p G two s", two=2, s=GRP // 2
            )
            nc.vector.tensor_tensor(zs[:], zav[:, :, 0], zav[:, :, 1], op=OP.add)
            zs2 = st_pool.tile([P, T * NG, GRP // 4], BF16, name="zs2")
            zsv = zs[:].rearrange("p G (two s) -> p G two s", two=2)
            nc.vector.tensor_tensor(zs2[:], zsv[:, :, 0], zsv[:, :, 1], op=OP.add)
            sa = st_pool.tile([P, T * NG], F32, name="sa")
            nc.vector.tensor_reduce(
                sa[:], zs2[:], axis=mybir.AxisListType.X, op=OP.add
            )
            ra = st_pool.tile([P, T * NG], F32, name="ra")
            nc.vector.reciprocal(ra[:], sa[:])
            # expand ra -> [P, 8, GRP] bf16 via a stride-0 copy fan-out chain
            # (steps 2 and 3 have packed inner runs, so they hit fast modes)
            r2 = st_pool.tile([P, T * NG, 2], BF16, name="r2")
            nc.vector.tensor_copy(
                r2[:],
                ra[:].rearrange("p (G one) -> p G one", one=1)
                .broadcast_to([P, T * NG, 2]),
            )
            raw = st_pool.tile([P, T * NG, GRP], BF16, name="raw")
            nc.vector.tensor_copy(
                raw[:].rearrange("p G (f r) -> p G f r", r=2),
                r2[:].rearrange("p G (one r) -> p G one r", one=1)
                .broadcast_to([P, T * NG, GRP // 2, 2]),
            )

            # --- product: w1 = bhB.T (<=1), w2 = w1*za, out = w2*raw --------
            w1 = w1_pool.tile([P, T, B], BF16)
            nc.vector.tensor_tensor(
                w1[:].rearrange("p (g u) b -> p g u b", g=NG),
                zbt.rearrange("p (g u) b -> p g u b", g=NG),
                rbb[:].rearrange("p g (one b) -> p g one b", one=1)
                .broadcast_to([P, NG, T // NG, B]),
                op=OP.mult,
            )
            w2 = w2_pool.tile([P, T, B], BF16)
            nc.vector.tensor_tensor(w2[:], w1[:], za, op=OP.mult)
            o_sb = o_pool.tile([P, T, B], BF16)
            nc.vector.tensor_tensor(
                o_sb[:].rearrange("p t b -> p (t b)"),
                w2[:].rearrange("p t b -> p (t b)"),
                raw[:].rearrange("p G s -> p (G s)"),
                op=OP.mult,
            )
            nc.sync.dma_start(o[k], o_sb[:])

    nc.compile()
    return nc


_NC = None


def _get_nc():
    global _NC
    if _NC is None:
        _NC = build()
    return _NC


def _to_pmajor(blocks: np.ndarray) -> np.ndarray:
    # (n, 512, 512) row-major -> (n, 128, 4, 512): row r = t*P + p lands at
    # [p, t, :], so every SBUF partition's bytes are contiguous in DRAM.
    n = blocks.shape[0]
    return np.ascontiguousarray(
        blocks.reshape(n, T, P, B).transpose(0, 2, 1, 3)
    )


def make_in_maps(sims: np.ndarray) -> list[dict[str, np.ndarray]]:
    in_maps = []
    for c in range(NCORES):
        a_stack = np.empty((NSLOTS, B, B), np.float16)
        bt_stack = np.empty((NSLOTS, B, B), np.float16)
        for k, (i, j) in enumerate(CORE_PAIRS[c]):
            ablk = sims[i * B:(i + 1) * B, j * B:(j + 1) * B].astype(np.float16)
            if i == j:
                np.fill_diagonal(ablk, MASK)
            a_stack[k] = ablk
            if i == j:
                bt_stack[k] = ablk.T
            else:
                bt_stack[k] = (
                    sims[j * B:(j + 1) * B, i * B:(i + 1) * B]
                    .astype(np.float16).T
                )
        from ml_dtypes import bfloat16

        sel = np.zeros((NG, NG, P), bfloat16)
        for g in range(NG):
            sel[g, g, :] = 1.0
        # ab[k, p, 0] = A rows, ab[k, p, 1] = B.T rows (partition-major)
        ab = np.ascontiguousarray(
            np.stack([_to_pmajor(a_stack), _to_pmajor(bt_stack)], axis=2)
        )
        in_maps.append({"ab": ab, "selc": sel})
    return in_maps


def assemble(results: list[dict[str, np.ndarray]]) -> np.ndarray:
    out = np.empty((N, N), np.float32)
    for c in range(NCORES):
        o_pm = np.asarray(results[c]["o"], dtype=np.float32)
        o_stack = np.ascontiguousarray(
            o_pm.transpose(0, 2, 1, 3).reshape(NSLOTS, B, B)
        )
        for k, (i, j) in enumerate(CORE_PAIRS[c]):
            out[i * B:(i + 1) * B, j * B:(j + 1) * B] = o_stack[k]
            if i != j:
                out[j * B:(j + 1) * B, i * B:(i + 1) * B] = o_stack[k].T
    return out


def run_on_hw(sims: np.ndarray, **spmd_kwargs):
    """Run the kernel on the 8 NeuronCores. Returns (out, BassKernelResults).

    The device occasionally throws a transient NRT_EXEC_UNIT_UNRECOVERABLE
    and needs ~a minute to come back, so failed runs are retried."""
    import time

    nc = _get_nc()
    in_maps = make_in_maps(sims)
    last_exc = None
    for attempt in range(3):
        if attempt:
            time.sleep(75)
        try:
            res = run_bass_kernel_spmd(
                nc, in_maps, core_ids=list(range(NCORES)), **spmd_kwargs
            )
            return assemble(res.results), res
        except Exception as exc:  # noqa: BLE001 - device flake, retry
            last_exc = exc
    raise last_exc


def kernel(similarities: np.ndarray) -> np.ndarray:
    sims = np.ascontiguousarray(similarities, dtype=np.float32)
    assert sims.shape == (N, N)
    out, _ = run_on_hw(sims)
    return out


if __name__ == "__main__":
    rng = np.random.default_rng(0)
    sims = rng.standard_normal((N, N), dtype=np.float32)
    out = kernel(similarities=sims)
    print("out", out.shape, out.dtype, float(out.max()))
